# revision 11
# baseline (speedup 1.0000x reference)
"""BiLSTM-CRF loss kernel for Trainium2 (8 NeuronCores, Bass/Tile) — v4.

Pair scheme: cores 0-3 run the FWD direction for seqs [16c:16c+16],
cores 4-7 the BWD direction for the same seqs (host feeds time-reversed
x / labels / role-swapped CRF tensors, so the program is SPMD-uniform).

v4 changes vs v3:
 - Whh stationary is fp8e3m4 (host-prescaled x32 on top of the x2
   g-row tanh fold; DVE folds the 1/32 back in) while h stays bf16 —
   FWL loads a 128-col fp8 stationary in ~32 cycles vs 64 for bf16,
   and the recurrence is LDWEIGHTS-bound at N=16.
 - The per-step GX injection (identity matmul into PSUM) is gone; the
   gate PSUM holds pure Whh.h and a DVE scalar_tensor_tensor computes
   gp/32 + gx before the sigmoid. Saves ~2 PE ops/step.
 - Each gate half accumulates into its own full-bank PSUM tile so the
   H0 PSUM read (DVE/ACT) overlaps the H1 matmul stream.
 - CRF alpha/beta split: each core scans only 256 steps of its own
   orientation (fwd = alpha half, bwd = beta half via the transposed
   recursion), exchanges only the second half of its emissions plus
   the final alpha/logacc, and the pair combines
   Z = alpha_255^T E alphahat_255.  Numerator is split between the
   pair (each side takes its first-half em terms + its start bonus;
   the shared boundary transition is half-weighted on both).
"""

import sys

sys.path.insert(0, "/opt/trn_rl_repo")

import numpy as np
import ml_dtypes
from contextlib import ExitStack

import concourse.bass as bass
import concourse.bacc as bacc
import concourse.tile as tile
import concourse.mybir as mybir

F32 = mybir.dt.float32
BF16 = mybir.dt.bfloat16
FP8 = mybir.dt.float8e4
FP8E3 = mybir.dt.float8e3
I32 = mybir.dt.int32
PM = mybir.MatmulPerfMode
AFT = mybir.ActivationFunctionType
ALU = mybir.AluOpType
AXL = mybir.AxisListType

NCORES = 8
NPAIR = 4
WHH_SCL = 32.0  # host prescale on Whh (fp8e3m4 grid); DVE divides back


def build_program(b, S, E, HD, T, B_full, R=12, AW=32, ECH=8):
    KE = E // 128
    NH = HD // 128          # 4 k-tiles of h
    NM = 4 * NH             # 16 gate m-tiles, khalf-major perm order
    W = NH * b              # h width per step (64 for b=16)
    WH = W // 2             # per-half h width (32)
    SB = S * b
    S2 = S // 2
    SB2 = S2 * b
    H4 = 4 * HD
    NWIN = S // AW
    T1 = T + 1              # alpha + logacc rows
    assert S % AW == 0 and AW % ECH == 0

    nc = bacc.Bacc("TRN2", target_bir_lowering=False, debug=False,
                   num_devices=NCORES)

    KE2 = KE // 2
    xT = nc.dram_tensor("xT", [KE2, 128, 2, SB], FP8, kind="ExternalInput")
    wihT = nc.dram_tensor("wihT", [KE2, 128, 2, H4], FP8, kind="ExternalInput")
    whhT = nc.dram_tensor("whhT", [NH, 128, H4], FP8E3, kind="ExternalInput")
    bias4 = nc.dram_tensor("bias4", [128, NM], F32, kind="ExternalInput")
    wtagT = nc.dram_tensor("wtagT", [NH, 128, T], BF16, kind="ExternalInput")
    tagb = nc.dram_tensor("tagb", [T, 1], F32, kind="ExternalInput")
    labT = nc.dram_tensor("labT", [S, b], I32, kind="ExternalInput")
    transR = nc.dram_tensor("transR", [T, T], F32, kind="ExternalInput")
    transRT = nc.dram_tensor("transRT", [T, T], F32, kind="ExternalInput")
    svR = nc.dram_tensor("svR", [T, 1], F32, kind="ExternalInput")
    m0 = nc.dram_tensor("m0", [T, 1], F32, kind="ExternalInput")
    m1 = nc.dram_tensor("m1", [T, 1], F32, kind="ExternalInput")
    loss = nc.dram_tensor("loss", [1, 1], F32, kind="ExternalOutput")

    with tile.TileContext(nc) as tc, ExitStack() as top:
        dram = top.enter_context(tc.tile_pool(name="dram", bufs=1, space="DRAM"))
        emdr2 = dram.tile([T, SB2], BF16)
        emdb = dram.tile([T, SB2], F32)
        emdbo = dram.tile([T, SB2], F32)
        albdb = dram.tile([T, 4 * b], F32)
        albdbo = dram.tile([T, 4 * b], F32)
        lossdb = dram.tile([1, 1], F32)
        lossout = dram.tile([1, 1], F32)

        persist = top.enter_context(tc.tile_pool(name="persist", bufs=1))
        wtag_sb = persist.tile([128, NH * T], BF16)
        nc.sync.dma_start(wtag_sb[:], wtagT[:])
        tagb_sb = persist.tile([T, 1], F32)
        nc.sync.dma_start(tagb_sb[:], tagb[:])
        em_self = persist.tile([T, SB], BF16)

        # ------------------------- fused LSTM loop -------------------------
        lstm = ExitStack()
        wpool = lstm.enter_context(tc.tile_pool(name="wpool", bufs=1))
        wih_sb = wpool.tile([128, KE2 * 2 * H4], FP8)
        nc.sync.dma_start(wih_sb[:], wihT[:])
        whh_sb = wpool.tile([128, NH * H4], FP8E3)
        nc.sync.dma_start(whh_sb[:], whhT[:])
        bias_sb = wpool.tile([128, NM], F32)
        nc.sync.dma_start(bias_sb[:], bias4[:])

        xwin = lstm.enter_context(tc.tile_pool(name="xwin", bufs=2))
        gxwin = lstm.enter_context(tc.tile_pool(name="gxwin", bufs=2))
        apsum = lstm.enter_context(tc.tile_pool(name="apsum", bufs=2,
                                                space="PSUM"))
        # one full PSUM bank per (half, parity) so PSUM reads of one
        # half overlap the other half's matmul stream
        gpsumA = lstm.enter_context(tc.tile_pool(name="gpsumA", bufs=2,
                                                 space="PSUM"))
        gpsumB = lstm.enter_context(tc.tile_pool(name="gpsumB", bufs=2,
                                                 space="PSUM"))
        empsum = lstm.enter_context(tc.tile_pool(name="empsum", bufs=2,
                                                 space="PSUM"))
        hpool = lstm.enter_context(tc.tile_pool(name="hpool", bufs=2))
        spool = lstm.enter_context(tc.tile_pool(name="spool", bufs=2))
        cpool = lstm.enter_context(tc.tile_pool(name="cpool", bufs=1))
        tpool = lstm.enter_context(tc.tile_pool(name="tpool", bufs=2))

        ct = [cpool.tile([128, WH], F32, tag=f"c{h}", name=f"ct{h}")
              for h in range(2)]

        def dma_xwin(w):
            t = xwin.tile([128, KE2 * 2 * AW * b], FP8, tag="xw",
                          name=f"xw_{w}")
            nc.sync.dma_start(t[:],
                             xT[:, :, :, w * AW * b:(w + 1) * AW * b])
            return t

        def a_work(w, xw, gxt):
            wv = wih_sb[:].rearrange("p (k q c) -> p k q c", k=KE2, q=2)
            xv = xw[:].rearrange("p (k q c) -> p k q c", k=KE2, q=2)
            for mm in range(NM):
                ap = apsum.tile([128, AW * b], F32, tag="aps",
                                name=f"aps_{w}_{mm}")
                for ke in range(KE2):
                    yield lambda ap=ap, mm=mm, ke=ke, xv=xv, wv=wv: \
                        nc.tensor.matmul(
                            ap[:],
                            wv[:, ke, :, mm * 128:(mm + 1) * 128],
                            xv[:, ke, :, :],
                            start=(ke == 0), stop=(ke == KE2 - 1),
                            perf_mode=PM.DoubleRow)
                yield lambda ap=ap, mm=mm, gxt=gxt: \
                    nc.vector.tensor_scalar_add(
                        gxt[:].rearrange("p (t m c) -> p t m c",
                                         t=AW, m=NM)[:, :, mm, :],
                        ap[:].rearrange("p (t c) -> p t c", t=AW),
                        bias_sb[:, mm:mm + 1])

        xw_t = dma_xwin(0)
        gx_t = gxwin.tile([128, NM * AW * b], BF16, tag="gx", name="gx_0")
        for job in a_work(0, xw_t, gx_t):
            job()
        xw_next = gx_next = None

        pending = []
        h_prev = None
        hch = None
        GB = 8 * b  # gate cols per half (128)
        for tau in range(S):
            w, sw = tau // AW, tau % AW
            if sw == 0:
                if w + 1 < NWIN:
                    xw_next = dma_xwin(w + 1)
                    gx_next = gxwin.tile([128, NM * AW * b], BF16,
                                         tag="gx", name=f"gx_{w + 1}")
                    pending = list(a_work(w + 1, xw_next, gx_next))
                    pending.reverse()
                else:
                    pending = []
            quota = ((len(pending) + (AW - sw) - 1) // (AW - sw)
                     if pending else 0)

            gxv = gx_t[:].rearrange("p (t m c) -> p t m c",
                                    t=AW, m=NM)[:, sw, :, :]
            if tau % ECH == 0:
                hch = hpool.tile([128, ECH * W], BF16, tag="h",
                                 name=f"hch_{tau}")
            toff = (tau % ECH) * W
            for H in range(2):
                pool = gpsumA if H == 0 else gpsumB
                gxh = gxv[:, H * 8:(H + 1) * 8, :]
                if tau > 0:
                    gp = pool.tile([128, 512], F32, tag=f"g{H}",
                                   name=f"gp_{H}_{tau}")
                    # PE: pure Whh.h recurrence accumulation (x32 scale)
                    first = True
                    for kh in range(2):          # input k-half
                        for mm in range(H * 8, (H + 1) * 8):
                            for kt in (2 * kh, 2 * kh + 1):
                                mloc = mm - H * 8
                                nc.tensor.matmul(
                                    gp[:, mloc * b:(mloc + 1) * b],
                                    whh_sb[:, kt * H4 + mm * 128:
                                           kt * H4 + (mm + 1) * 128],
                                    h_prev[:, kt * b:(kt + 1) * b],
                                    start=first,
                                    stop=(mm == (H + 1) * 8 - 1 and
                                          kt == 2 * kh + 1),
                                    skip_group_check=True)
                                first = False

                if H == 0:
                    for _ in range(quota):
                        if pending:
                            pending.pop()()

                sg = spool.tile([128, GB], F32, tag=f"sig{H}",
                                name=f"sg_{H}_{tau}")
                if tau == 0:
                    nc.scalar.activation(
                        sg[:].rearrange("p (m c) -> p m c", m=8),
                        gxh, AFT.Sigmoid)
                else:
                    # gp/32 + gx, in place in the PSUM bank, then sigmoid
                    nc.vector.scalar_tensor_tensor(
                        gp[:, 0:GB].rearrange("p (m c) -> p m c", m=8),
                        gp[:, 0:GB].rearrange("p (m c) -> p m c", m=8),
                        1.0 / WHH_SCL, gxh,
                        op0=ALU.mult, op1=ALU.add)
                    nc.scalar.activation(sg[:], gp[:, 0:GB], AFT.Sigmoid)

                def _v2(out):
                    # fused (sig_g2 - 0.5) * sig_i always on DVE (Pool
                    # lacks scalar_tensor_tensor and is slow on 2-op seqs)
                    nc.vector.scalar_tensor_tensor(
                        out, sg[:, 3 * WH:4 * WH], 0.5, sg[:, 0:WH],
                        op0=ALU.subtract, op1=ALU.mult)
                if tau == 0:
                    _v2(ct[H][:])
                else:
                    v2 = tpool.tile([128, WH], F32, tag=f"v2{H}",
                                    name=f"v2_{H}_{tau}")
                    _v2(v2[:])
                    wt = tpool.tile([128, WH], F32, tag=f"w{H}",
                                    name=f"wt_{H}_{tau}")
                    nc.vector.tensor_tensor(wt[:], sg[:, WH:2 * WH],
                                            ct[H][:], op=ALU.mult)
                    nc.vector.tensor_tensor(ct[H][:], v2[:], wt[:],
                                            op=ALU.add)
                tca = tpool.tile([128, WH], F32, tag=f"tanc{H}",
                                 name=f"tca_{H}_{tau}")
                nc.scalar.activation(tca[:], ct[H][:], AFT.Tanh, scale=2.0)
                # h half H -> k-tiles 2H,2H+1 = cols [H*WH : H*WH+WH]
                nc.vector.tensor_tensor(
                    hch[:, toff + H * WH:toff + (H + 1) * WH],
                    sg[:, 2 * WH:3 * WH], tca[:], op=ALU.mult)
            h_prev = hch[:, toff:toff + W]

            if tau % ECH == ECH - 1:
                ep = empsum.tile([T, ECH * b], F32, tag="emps",
                                 name=f"ep_{tau}")
                hv = hch[:].rearrange("p (t k c) -> p t k c", t=ECH, k=NH)
                for kt in range(NH):
                    nc.tensor.matmul(
                        ep[:].rearrange("p (t c) -> p t c", t=ECH),
                        wtag_sb[:, kt * T:(kt + 1) * T], hv[:, :, kt, :],
                        start=(kt == 0), stop=(kt == NH - 1))
                emv = em_self[:].rearrange(
                    "j (t c) -> j t c", t=S)[:, tau - ECH + 1:tau + 1, :]
                nc.vector.tensor_copy(
                    emv, ep[:].rearrange("p (t c) -> p t c", t=ECH))
            if sw == AW - 1 and gx_next is not None:
                xw_t, gx_t = xw_next, gx_next
                xw_next = gx_next = None

        lstm.close()

        # ---------------- em exchange (pair AllReduce, half S) --------------
        # Each core ships its SECOND-half-of-tau emissions reversed into
        # p-order; buf[p] then holds my_em[511-p] + partner_em[511-p'],
        # and partner_em[511-p'] is exactly the complement of my tau=p.
        with ExitStack() as ph:
            big = ph.enter_context(tc.tile_pool(name="exbig", bufs=1))
            nc.sync.dma_start(emdr2[:], em_self[:, SB2:])
            rev2 = big.tile([T, SB2], BF16, tag="bigF", name="rev2")
            nc.sync.dma_start(
                rev2[:], emdr2[:].rearrange("j (t c) -> j t c",
                                            t=S2)[:, ::-1, :])
            slot = big.tile([T, SB2], F32, tag="bigB", name="slot")
            nc.vector.tensor_copy(slot[:], rev2[:])
            nc.sync.dma_start(emdb[:], slot[:])
            nc.gpsimd.collective_compute(
                "AllReduce", ALU.add,
                replica_groups=[[c, c + NPAIR] for c in range(NPAIR)],
                ins=[emdb.opt()], outs=[emdbo.opt()])
            t0 = big.tile([T, SB2], F32, tag="bigA", name="t0")
            nc.sync.dma_start(t0[:], emdbo[:])
            # em (first half, my orientation) = local + buf - rev2 + b_tag
            em1 = big.tile([T, SB2], F32, tag="bigD", name="em1")
            nc.vector.tensor_tensor(em1[:], t0[:], rev2[:], op=ALU.subtract)
            nc.vector.tensor_tensor(em1[:], em1[:], em_self[:, 0:SB2],
                                    op=ALU.add)
            nc.vector.tensor_scalar_add(em1[:], em1[:], tagb_sb[:])

            # ------------------------- CRF tail -------------------------
            sp = ph.enter_context(tc.tile_pool(name="crftmp", bufs=2))
            ap_ = ph.enter_context(tc.tile_pool(name="alphas", bufs=3))
            pp = ph.enter_context(tc.tile_pool(name="crfps", bufs=1,
                                               space="PSUM"))

            cst = sp.tile([T, T], F32, tag="cst")
            nc.sync.dma_start(cst[:], transR[:])
            cstT = sp.tile([T, T], F32, tag="cstT")
            nc.sync.dma_start(cstT[:], transRT[:])
            st_sb = sp.tile([T, 1], F32, tag="stv")
            nc.sync.dma_start(st_sb[:], svR[:])
            m0_sb = sp.tile([T, 1], F32, tag="m0v")
            nc.sync.dma_start(m0_sb[:], m0[:])
            m1_sb = sp.tile([T, 1], F32, tag="m1v")
            nc.sync.dma_start(m1_sb[:], m1[:])

            # numerator partial: my first-half em terms + start bonus +
            # transition pairs tau in [0,255], boundary pair half-weighted
            NLB = S2 + 1  # label steps needed: 0..256
            lab9 = big.tile([T, NLB * b], I32, tag="bigA", name="lab9")
            nc.sync.dma_start(
                lab9[:],
                labT[:].rearrange("s c -> (s c)")[None, 0:NLB * b]
                .broadcast_to((T, NLB * b)))
            io9 = sp.tile([T, 1], I32, tag="io9")
            nc.gpsimd.iota(io9[:], pattern=[[0, 1]], base=0,
                           channel_multiplier=1)
            io9f = sp.tile([T, 1], F32, tag="io9f")
            nc.gpsimd.tensor_copy(io9f[:], io9[:])
            labf = big.tile([T, NLB * b], F32, tag="bigC", name="labf")
            nc.gpsimd.tensor_copy(labf[:], lab9[:])
            onehot = big.tile([T, NLB * b], F32, tag="bigE", name="onehot")
            nc.gpsimd.tensor_scalar(onehot[:], labf[:], io9f[:], None,
                                    op0=ALU.is_equal)
            gmul = big.tile([T, SB2], F32, tag="bigA", name="gmul")
            nc.gpsimd.tensor_tensor(gmul[:], onehot[:, 0:SB2], em1[:],
                                    op=ALU.mult)
            acc = sp.tile([T, b], F32, tag="acc")
            nc.vector.tensor_reduce(
                acc[:], gmul[:].rearrange("j (t c) -> j c t", c=b),
                op=ALU.add, axis=AXL.X)
            stsc = sp.tile([T, b], F32, tag="stsc")
            nc.gpsimd.tensor_scalar_mul(stsc[:], onehot[:, 0:b], st_sb[:])
            nc.gpsimd.tensor_add(acc[:], acc[:], stsc[:])
            for tc0 in range(0, S2, 32):
                tn = min(32, S2 - tc0)
                thp = pp.tile([T, 32 * b], F32, tag="thp", bufs=2,
                              name=f"thp_{tc0}")
                nc.tensor.matmul(thp[:, 0:tn * b], cst[:],
                                 onehot[:, tc0 * b:(tc0 + tn) * b],
                                 start=True, stop=True)
                v = sp.tile([T, 32 * b], F32, tag="v")
                nc.vector.tensor_mul(v[:, 0:tn * b], thp[:, 0:tn * b],
                                     onehot[:, (tc0 + 1) * b:(tc0 + 1 + tn) * b])
                vr = sp.tile([T, b], F32, tag="vr")
                nc.vector.tensor_reduce(
                    vr[:], v[:, 0:tn * b].rearrange("j (t c) -> j c t", c=b),
                    op=ALU.add, axis=AXL.X)
                nc.vector.tensor_add(acc[:], acc[:], vr[:])
            # boundary pair (tau = S2-1) was fully counted on both cores:
            # subtract half of it here (symmetric on the pair)
            thb = pp.tile([T, b], F32, tag="thp", bufs=2, name="thb")
            nc.tensor.matmul(thb[:], cst[:],
                             onehot[:, (S2 - 1) * b:S2 * b],
                             start=True, stop=True)
            vb = sp.tile([T, b], F32, tag="vb")
            nc.vector.tensor_mul(vb[:], thb[:], onehot[:, S2 * b:(S2 + 1) * b])
            nc.vector.tensor_scalar_mul(vb[:], vb[:], -0.5)
            nc.vector.tensor_add(acc[:], acc[:], vb[:])
            ones9 = sp.tile([T, 1], F32, tag="ones9")
            nc.vector.memset(ones9[:], 1.0)
            ones19 = sp.tile([1, T], F32, tag="ones19")
            nc.vector.memset(ones19[:], 1.0)
            nump = pp.tile([1, b], F32, tag="nump")
            nc.tensor.matmul(nump[:], ones9[:], acc[:], start=True, stop=True)
            num_sb = sp.tile([1, b], F32, tag="num")
            nc.vector.tensor_copy(num_sb[:], nump[:])

            # partition half-scan: probability domain, my orientation
            Em = sp.tile([T, T], F32, tag="Em")
            nc.scalar.activation(Em[:], cst[:], AFT.Exp)
            EmT = sp.tile([T, T], F32, tag="EmT")
            nc.scalar.activation(EmT[:], cstT[:], AFT.Exp)
            eem = big.tile([T, SB2], F32, tag="bigC", name="eem")
            nc.scalar.activation(eem[:], em1[:], AFT.Exp)
            es = sp.tile([T, 1], F32, tag="es")
            nc.scalar.activation(es[:], st_sb[:], AFT.Exp)
            logacc = sp.tile([1, b], F32, tag="logacc")
            nc.vector.memset(logacc[:], 0.0)
            alpha = ap_.tile([T, b], F32, tag="alpha", name="alpha_init")
            nc.vector.tensor_scalar_mul(alpha[:], eem[:, 0:b], es[:])
            for t in range(1, S2):
                aps = pp.tile([T, b], F32, tag="aps", bufs=2,
                              name=f"apsn_{t}")
                nc.tensor.matmul(aps[:], Em[:], alpha[:], start=True, stop=True)
                alpha = ap_.tile([T, b], F32, tag="alpha", name=f"alpha_{t}")
                nc.vector.tensor_mul(alpha[:], aps[:], eem[:, t * b:(t + 1) * b])
                if t % R == 0 or t == S2 - 1:
                    ssum = pp.tile([1, b], F32, tag="ssum")
                    nc.tensor.matmul(ssum[:], ones9[:], alpha[:],
                                     start=True, stop=True)
                    ls = sp.tile([1, b], F32, tag="ls")
                    nc.scalar.activation(ls[:], ssum[:], AFT.Ln)
                    nc.vector.tensor_add(logacc[:], logacc[:], ls[:])
                    rc = sp.tile([1, b], F32, tag="rc")
                    nc.vector.reciprocal(rc[:], ssum[:])
                    bc = pp.tile([T, b], F32, tag="bc")
                    nc.tensor.matmul(bc[:], ones19[:], rc[:],
                                     start=True, stop=True)
                    a2 = ap_.tile([T, b], F32, tag="alpha", name=f"a2_{t}")
                    nc.vector.tensor_mul(a2[:], alpha[:], bc[:])
                    alpha = a2

            # ---------------- alpha exchange (pair AllReduce) ----------------
            # column-block layout (no partition shifts): cols [0:b] fwd
            # alpha, [b:2b] bwd alpha, [2b:3b] row0 = fwd logacc,
            # [3b:4b] row0 = bwd logacc
            slot4 = sp.tile([T, 4 * b], F32, tag="sl4")
            nc.vector.memset(slot4[:], 0.0)
            nc.vector.tensor_scalar_mul(slot4[:, 0:b], alpha[:], m0_sb[:])
            nc.vector.tensor_scalar_mul(slot4[:, b:2 * b], alpha[:], m1_sb[:])
            nc.vector.tensor_scalar_mul(slot4[0:1, 2 * b:3 * b], logacc[:],
                                        m0_sb[0:1, :])
            nc.vector.tensor_scalar_mul(slot4[0:1, 3 * b:4 * b], logacc[:],
                                        m1_sb[0:1, :])
            nc.sync.dma_start(albdb[:], slot4[:])
            nc.gpsimd.collective_compute(
                "AllReduce", ALU.add,
                replica_groups=[[c, c + NPAIR] for c in range(NPAIR)],
                ins=[albdb.opt()], outs=[albdbo.opt()])
            buf4 = sp.tile([T, 4 * b], F32, tag="bf4")
            nc.sync.dma_start(buf4[:], albdbo[:])
            # partner alpha/acc: pick the opposite role's column block
            pa = sp.tile([T, b], F32, tag="pa")
            nc.vector.tensor_scalar_mul(pa[:], buf4[:, 0:b], m1_sb[:])
            pb = sp.tile([T, b], F32, tag="pb")
            nc.vector.tensor_scalar_mul(pb[:], buf4[:, b:2 * b], m0_sb[:])
            nc.vector.tensor_add(pa[:], pa[:], pb[:])
            pacc = sp.tile([1, b], F32, tag="pacc")
            nc.vector.tensor_scalar_mul(pacc[:], buf4[0:1, 2 * b:3 * b],
                                        m1_sb[0:1, :])
            pacc2 = sp.tile([1, b], F32, tag="pacc2")
            nc.vector.tensor_scalar_mul(pacc2[:], buf4[0:1, 3 * b:4 * b],
                                        m0_sb[0:1, :])
            nc.vector.tensor_add(pacc[:], pacc[:], pacc2[:])

            # Z = alpha_mine^T . exp(transR) . alpha_partner
            vps = pp.tile([T, b], F32, tag="bc", name="vps")
            nc.tensor.matmul(vps[:], EmT[:], pa[:], start=True, stop=True)
            wz = sp.tile([T, b], F32, tag="wz")
            nc.vector.tensor_mul(wz[:], alpha[:], vps[:])
            zp = pp.tile([1, b], F32, tag="ssum")
            nc.tensor.matmul(zp[:], ones9[:], wz[:], start=True, stop=True)
            lz = sp.tile([1, b], F32, tag="lz")
            nc.scalar.activation(lz[:], zp[:], AFT.Ln)
            logz = sp.tile([1, b], F32, tag="logz")
            nc.vector.tensor_add(logz[:], lz[:], logacc[:])
            nc.vector.tensor_add(logz[:], logz[:], pacc[:])
            nc.vector.tensor_scalar_mul(logz[:], logz[:], 0.5)
            lv = sp.tile([1, b], F32, tag="lv")
            nc.vector.tensor_sub(lv[:], num_sb[:], logz[:])
            tot = sp.tile([1, 1], F32, tag="tot")
            nc.vector.tensor_reduce(tot[:], lv[:], op=ALU.add, axis=AXL.X)
            sc = sp.tile([1, 1], F32, tag="sc")
            nc.vector.tensor_scalar_mul(sc[:], tot[:], -1.0 / B_full)
            nc.sync.dma_start(lossdb[:], sc[:])
            nc.gpsimd.collective_compute(
                "AllReduce", ALU.add,
                replica_groups=[list(range(NCORES))],
                ins=[lossdb.opt()], outs=[lossout.opt()])
            lf = sp.tile([1, 1], F32, tag="lf")
            nc.sync.dma_start(lf[:], lossout[:])
            nc.sync.dma_start(loss[:], lf[:])

    nc.compile()
    return nc


# ---------------------------------------------------------------------------
# host-side sharding
# ---------------------------------------------------------------------------

def _perm_khalf(HD):
    """Gate-dim perm: torch (i,f,g,o) -> khalf-major (i,f,o,g)x2 halves."""
    base = {"i": 0, "f": HD, "g": 2 * HD, "o": 3 * HD}
    idx = []
    half = HD // 2
    for H in range(2):
        for gate in ("i", "f", "o", "g"):
            lo = base[gate] + H * half
            idx.append(np.arange(lo, lo + half))
    return np.concatenate(idx)


def shard_inputs(inputs, b, S, E, HD, T):
    KE, NH = E // 128, HD // 128
    NM = 4 * NH
    T1 = T + 1
    perm = _perm_khalf(HD)
    gmask = np.zeros(4 * HD, np.float32)  # rows holding g-gate blocks
    half = HD // 2
    for H in range(2):
        lo = (H * 4 + 3) * half
        gmask[lo:lo + half] = 1.0
    bf = ml_dtypes.bfloat16
    f8 = ml_dtypes.float8_e4m3fn
    f8e3 = ml_dtypes.float8_e3m4
    KE2 = KE // 2
    x = np.asarray(inputs["x"], np.float32)[:, :S]
    labels = np.asarray(inputs["labels"]).astype(np.int32)[:, :S]
    trans = np.asarray(inputs["transitions"], np.float32)
    startv = np.asarray(inputs["start_trans"], np.float32).reshape(T, 1)
    endv = np.asarray(inputs["end_trans"], np.float32).reshape(T, 1)
    Wtag = np.asarray(inputs["W_tag"], np.float32)
    btag = np.asarray(inputs["b_tag"], np.float32).reshape(T, 1)

    per_dir = {}
    for d, sfx in enumerate(("f", "b")):
        Wih = np.asarray(inputs[f"W_ih_{sfx}"], np.float32)[perm].copy()
        Whh = np.asarray(inputs[f"W_hh_{sfx}"], np.float32)[perm].copy()
        bias = (np.asarray(inputs[f"b_ih_{sfx}"], np.float32)
                + np.asarray(inputs[f"b_hh_{sfx}"], np.float32))[perm].copy()
        scale = 1.0 + gmask  # 2x on g rows (tanh -> sigmoid fold)
        Wih *= scale[:, None]
        Whh *= scale[:, None]
        bias *= scale
        per_dir[d] = dict(
            wihT=np.ascontiguousarray(
                Wih.T.reshape(KE2, 2, 128, 4 * HD).transpose(0, 2, 1, 3)
            ).astype(f8),
            whhT=np.ascontiguousarray(
                (Whh * WHH_SCL).T.reshape(NH, 128, 4 * HD)).astype(f8e3),
            bias4=np.ascontiguousarray(
                bias.reshape(NM, 128).T).astype(np.float32),
            wtagT=np.ascontiguousarray(
                Wtag[:, d * HD:(d + 1) * HD].T.reshape(NH, 128, T)).astype(bf),
            transR=(trans if d == 0
                    else np.ascontiguousarray(trans.T)),
            transRT=(np.ascontiguousarray(trans.T) if d == 0 else trans),
            svR=startv if d == 0 else endv,
            m0=np.full((T, 1), 1.0 - d, np.float32),
            m1=np.full((T, 1), float(d), np.float32),
        )

    in_maps = []
    for c in range(NCORES):
        d = c // NPAIR
        g = c % NPAIR
        xs = x[g * b:(g + 1) * b]
        lab = labels[g * b:(g + 1) * b]
        if d == 1:
            xs = xs[:, ::-1, :]
            lab = lab[:, ::-1]
        m = dict(per_dir[d])
        m["xT"] = np.ascontiguousarray(
            xs.transpose(2, 1, 0).reshape(KE2, 2, 128, S * b)
            .transpose(0, 2, 1, 3)).astype(f8)
        m["labT"] = np.ascontiguousarray(lab.T)
        m["tagb"] = btag
        in_maps.append(m)
    return in_maps


# ---------------------------------------------------------------------------
# entry point
# ---------------------------------------------------------------------------

_B, _S, _E, _HD, _T = 64, 512, 1024, 512, 9
_cache = {}


def _get_program(S=_S):
    if S not in _cache:
        _cache[S] = build_program(_B // NPAIR, S, _E, _HD, _T, _B)
    return _cache[S]


def kernel(**inputs) -> np.ndarray:
    from concourse.bass_utils import run_bass_kernel_spmd
    nc = _get_program()
    in_maps = shard_inputs(inputs, _B // NPAIR, _S, _E, _HD, _T)
    res = run_bass_kernel_spmd(nc, in_maps, list(range(NCORES)))
    out = np.asarray(res.results[0]["loss"], np.float32).reshape(())
    return out


# revision 29
# speedup vs baseline: 1.0981x; 1.0981x over previous
"""BiLSTM-CRF loss kernel for Trainium2 (8 NeuronCores, Bass/Tile) — v9.

Pair scheme: cores 0-3 run the FWD direction for seqs [16c:16c+16],
cores 4-7 the BWD direction for the same seqs (host feeds time-reversed
x / labels / role-swapped CRF tensors, so the program is SPMD-uniform).

Changes vs the v3 baseline:
 - Whh stationary is fp8e3m4 (host-prescaled x32 on top of the x2
   g-row tanh fold) with h kept bf16 (mixed-dtype matmul): FWL loads a
   128-col fp8 stationary in ~32 cycles vs 64 for bf16, and the
   recurrence is LDWEIGHTS-bound at N=16 — measured 25 ns/MM.
 - No GX injection matmul: the gate PSUM holds pure 32*Whh.h and a DVE
   scalar_tensor_tensor computes gp/32 + gx in place before the
   sigmoid.
 - Each gate half accumulates into its own full PSUM bank so one
   half's PSUM reads overlap the other half's matmul stream.
 - The input-projection (a_work) jobs drain at the END of each step so
   their 700 ns DVE bias-adds stop head-of-line-blocking the cell
   chain on the DVE FIFO.
 - CRF alpha/beta split: each core scans only 255 steps of its own
   orientation (fwd = alpha half, bwd = beta half via the transposed
   recursion), exchanges only the second half of its emissions plus
   the final alpha/logacc, and the pair combines
   Z = alpha_255^T E alphahat_255.  Numerator is split between the
   pair (each side takes its first-half em terms + its start bonus;
   the shared boundary transition is half-weighted on both).
"""

import sys

sys.path.insert(0, "/opt/trn_rl_repo")

import numpy as np
import ml_dtypes
from contextlib import ExitStack

import concourse.bass as bass
import concourse.bacc as bacc
import concourse.tile as tile
import concourse.mybir as mybir

F32 = mybir.dt.float32
BF16 = mybir.dt.bfloat16
FP8 = mybir.dt.float8e4
FP8E3 = mybir.dt.float8e3
I32 = mybir.dt.int32
PM = mybir.MatmulPerfMode
AFT = mybir.ActivationFunctionType
ALU = mybir.AluOpType
AXL = mybir.AxisListType

NCORES = 8
NPAIR = 4
WHH_SCL = 32.0  # host prescale on Whh (fp8e3m4 grid); DVE divides back


def build_program(b, S, E, HD, T, B_full, R=20, AW=32, ECH=8):
    KE = E // 128
    NH = HD // 128          # 4 k-tiles of h
    NM = 4 * NH             # 16 gate m-tiles, khalf-major perm order
    W = NH * b              # h width per step (64 for b=16)
    WH = W // 2             # per-half h width (32)
    SB = S * b
    S2 = S // 2
    SB2 = S2 * b
    H4 = 4 * HD
    NWIN = S // AW
    T1 = T + 1              # alpha + logacc rows
    assert S % AW == 0 and AW % ECH == 0

    nc = bacc.Bacc("TRN2", target_bir_lowering=False, debug=False,
                   num_devices=NCORES)

    KE2 = KE // 2
    xT = nc.dram_tensor("xT", [KE2, 128, 2, SB], FP8, kind="ExternalInput")
    wihT = nc.dram_tensor("wihT", [KE2, 128, 2, H4], FP8, kind="ExternalInput")
    whhT = nc.dram_tensor("whhT", [NH, 128, H4], FP8E3, kind="ExternalInput")
    bias4 = nc.dram_tensor("bias4", [128, NM], F32, kind="ExternalInput")
    wtagT = nc.dram_tensor("wtagT", [NH, 128, T], BF16, kind="ExternalInput")
    tagb = nc.dram_tensor("tagb", [T, 1], F32, kind="ExternalInput")
    labT = nc.dram_tensor("labT", [S, b], I32, kind="ExternalInput")
    transR = nc.dram_tensor("transR", [T, T], F32, kind="ExternalInput")
    transRT = nc.dram_tensor("transRT", [T, T], F32, kind="ExternalInput")
    svR = nc.dram_tensor("svR", [T, 1], F32, kind="ExternalInput")
    m0 = nc.dram_tensor("m0", [T, 1], F32, kind="ExternalInput")
    m1 = nc.dram_tensor("m1", [T, 1], F32, kind="ExternalInput")
    loss = nc.dram_tensor("loss", [1, 1], F32, kind="ExternalOutput")
    dbg = nc.dram_tensor("dbg", [4, b], F32, kind="ExternalOutput")

    with tile.TileContext(nc) as tc, ExitStack() as top:
        dram = top.enter_context(tc.tile_pool(name="dram", bufs=1, space="DRAM"))
        emdr2 = dram.tile([T, SB2], BF16)
        emdb = dram.tile([T, SB2], F32)
        emdbo = dram.tile([T, SB2], F32)
        albdb = dram.tile([T, 4 * b], F32)
        albdbo = dram.tile([T, 4 * b], F32)
        lossdb = dram.tile([1, 1], F32)
        lossout = dram.tile([1, 1], F32)

        persist = top.enter_context(tc.tile_pool(name="persist", bufs=1))
        wtag_sb = persist.tile([128, NH * T], BF16)
        nc.sync.dma_start(wtag_sb[:], wtagT[:])
        tagb_sb = persist.tile([T, 1], F32)
        nc.sync.dma_start(tagb_sb[:], tagb[:])
        em_self = persist.tile([T, SB], BF16)

        # ------------------------- fused LSTM loop -------------------------
        lstm = ExitStack()
        wpool = lstm.enter_context(tc.tile_pool(name="wpool", bufs=1))
        wih_sb = wpool.tile([128, KE2 * 2 * H4], FP8)
        nc.sync.dma_start(wih_sb[:], wihT[:])
        whh_sb = wpool.tile([128, NH * H4], FP8E3)
        nc.sync.dma_start(whh_sb[:], whhT[:])
        bias_sb = wpool.tile([128, NM], F32)
        nc.sync.dma_start(bias_sb[:], bias4[:])

        xwin = lstm.enter_context(tc.tile_pool(name="xwin", bufs=2))
        gxwin = lstm.enter_context(tc.tile_pool(name="gxwin", bufs=2))
        apsum = lstm.enter_context(tc.tile_pool(name="apsum", bufs=2,
                                                space="PSUM"))
        # one full PSUM bank per (half, parity) so PSUM reads of one
        # half overlap the other half's matmul stream
        gpsumA = lstm.enter_context(tc.tile_pool(name="gpsumA", bufs=2,
                                                 space="PSUM"))
        gpsumB = lstm.enter_context(tc.tile_pool(name="gpsumB", bufs=2,
                                                 space="PSUM"))
        empsum = lstm.enter_context(tc.tile_pool(name="empsum", bufs=2,
                                                 space="PSUM"))
        hpool = lstm.enter_context(tc.tile_pool(name="hpool", bufs=2))
        spool = lstm.enter_context(tc.tile_pool(name="spool", bufs=2))
        cpool = lstm.enter_context(tc.tile_pool(name="cpool", bufs=1))
        tpool = lstm.enter_context(tc.tile_pool(name="tpool", bufs=2))

        ct = [cpool.tile([128, WH], F32, tag=f"c{h}", name=f"ct{h}")
              for h in range(2)]

        def dma_xwin(w):
            t = xwin.tile([128, KE2 * 2 * AW * b], FP8, tag="xw",
                          name=f"xw_{w}")
            nc.sync.dma_start(t[:],
                             xT[:, :, :, w * AW * b:(w + 1) * AW * b])
            return t

        def a_work(w, xw, gxt):
            wv = wih_sb[:].rearrange("p (k q c) -> p k q c", k=KE2, q=2)
            xv = xw[:].rearrange("p (k q c) -> p k q c", k=KE2, q=2)
            for mm in range(NM):
                ap = apsum.tile([128, AW * b], F32, tag="aps",
                                name=f"aps_{w}_{mm}")
                for ke in range(KE2):
                    yield lambda ap=ap, mm=mm, ke=ke, xv=xv, wv=wv: \
                        nc.tensor.matmul(
                            ap[:],
                            wv[:, ke, :, mm * 128:(mm + 1) * 128],
                            xv[:, ke, :, :],
                            start=(ke == 0), stop=(ke == KE2 - 1),
                            perf_mode=PM.DoubleRow)
                yield lambda ap=ap, mm=mm, gxt=gxt: \
                    nc.vector.tensor_scalar_add(
                        gxt[:].rearrange("p (t m c) -> p t m c",
                                         t=AW, m=NM)[:, :, mm, :],
                        ap[:].rearrange("p (t c) -> p t c", t=AW),
                        bias_sb[:, mm:mm + 1])

        xw_t = dma_xwin(0)
        gx_t = gxwin.tile([128, NM * AW * b], BF16, tag="gx", name="gx_0")
        for job in a_work(0, xw_t, gx_t):
            job()
        xw_next = gx_next = None

        pending = []
        h_prev = None
        hch = None
        GB = 8 * b  # gate cols per half (128)
        for tau in range(S):
            w, sw = tau // AW, tau % AW
            if sw == 0:
                if w + 1 < NWIN:
                    xw_next = dma_xwin(w + 1)
                    gx_next = gxwin.tile([128, NM * AW * b], BF16,
                                         tag="gx", name=f"gx_{w + 1}")
                    pending = list(a_work(w + 1, xw_next, gx_next))
                    pending.reverse()
                else:
                    pending = []
            quota = ((len(pending) + (AW - sw) - 1) // (AW - sw)
                     if pending else 0)

            gxv = gx_t[:].rearrange("p (t m c) -> p t m c",
                                    t=AW, m=NM)[:, sw, :, :]
            if tau % ECH == 0:
                hch = hpool.tile([128, ECH * W], BF16, tag="h",
                                 name=f"hch_{tau}")
            toff = (tau % ECH) * W
            for H in range(2):
                pool = gpsumA if H == 0 else gpsumB
                gxh = gxv[:, H * 8:(H + 1) * 8, :]
                sg = spool.tile([128, GB], F32, tag=f"sig{H}",
                                name=f"sg_{H}_{tau}")
                if tau == 0:
                    nc.scalar.activation(
                        sg[:].rearrange("p (m c) -> p m c", m=8),
                        gxh, AFT.Sigmoid)
                else:
                    gp = pool.tile([128, 512], F32, tag=f"g{H}",
                                   name=f"gp_{H}_{tau}")
                    first = True
                    for kh in range(2):          # input k-half
                        for mm in range(H * 8, (H + 1) * 8):
                            for kt in (2 * kh, 2 * kh + 1):
                                mloc = mm - H * 8
                                nc.tensor.matmul(
                                    gp[:, mloc * b:(mloc + 1) * b],
                                    whh_sb[:, kt * H4 + mm * 128:
                                           kt * H4 + (mm + 1) * 128],
                                    h_prev[:, kt * b:(kt + 1) * b],
                                    start=first,
                                    stop=(mm == (H + 1) * 8 - 1 and
                                          kt == 2 * kh + 1),
                                    skip_group_check=True)
                                first = False
                    # gp/32 + gx in place, then sigmoid (v4-proven)
                    nc.vector.scalar_tensor_tensor(
                        gp[:, 0:GB].rearrange("p (m c) -> p m c", m=8),
                        gp[:, 0:GB].rearrange("p (m c) -> p m c", m=8),
                        1.0 / WHH_SCL, gxh,
                        op0=ALU.mult, op1=ALU.add)
                    nc.scalar.activation(sg[:], gp[:, 0:GB], AFT.Sigmoid)

                def _v2(out):
                    # fused (sig_g2 - 0.5) * sig_i always on DVE (Pool
                    # lacks scalar_tensor_tensor and is slow on 2-op seqs)
                    nc.vector.scalar_tensor_tensor(
                        out, sg[:, 3 * WH:4 * WH], 0.5, sg[:, 0:WH],
                        op0=ALU.subtract, op1=ALU.mult)
                if tau == 0:
                    _v2(ct[H][:])
                else:
                    v2 = tpool.tile([128, WH], F32, tag=f"v2{H}",
                                    name=f"v2_{H}_{tau}")
                    _v2(v2[:])
                    wt = tpool.tile([128, WH], F32, tag=f"w{H}",
                                    name=f"wt_{H}_{tau}")
                    nc.vector.tensor_tensor(wt[:], sg[:, WH:2 * WH],
                                            ct[H][:], op=ALU.mult)
                    nc.vector.tensor_tensor(ct[H][:], v2[:], wt[:],
                                            op=ALU.add)
                tca = tpool.tile([128, WH], F32, tag=f"tanc{H}",
                                 name=f"tca_{H}_{tau}")
                nc.scalar.activation(tca[:], ct[H][:], AFT.Tanh, scale=2.0)
                # h half H -> k-tiles 2H,2H+1 = cols [H*WH : H*WH+WH]
                nc.vector.tensor_tensor(
                    hch[:, toff + H * WH:toff + (H + 1) * WH],
                    sg[:, 2 * WH:3 * WH], tca[:], op=ALU.mult)
            h_prev = hch[:, toff:toff + W]
            # a_work jobs drain at end of step so their DVE ops land
            # behind the cell chain (DVE queue is strict FIFO)
            for _ in range(quota):
                if pending:
                    pending.pop()()

            if tau % ECH == ECH - 1:
                ep = empsum.tile([T, ECH * b], F32, tag="emps",
                                 name=f"ep_{tau}")
                hv = hch[:].rearrange("p (t k c) -> p t k c", t=ECH, k=NH)
                for kt in range(NH):
                    nc.tensor.matmul(
                        ep[:].rearrange("p (t c) -> p t c", t=ECH),
                        wtag_sb[:, kt * T:(kt + 1) * T], hv[:, :, kt, :],
                        start=(kt == 0), stop=(kt == NH - 1))
                emv = em_self[:].rearrange(
                    "j (t c) -> j t c", t=S)[:, tau - ECH + 1:tau + 1, :]
                nc.vector.tensor_copy(
                    emv, ep[:].rearrange("p (t c) -> p t c", t=ECH))
            if sw == AW - 1 and gx_next is not None:
                xw_t, gx_t = xw_next, gx_next
                xw_next = gx_next = None

        lstm.close()

        # ---------------- em exchange (pair AllReduce, half S) --------------
        # Each core ships its SECOND-half-of-tau emissions reversed into
        # p-order; buf[p] then holds my_em[511-p] + partner_em[511-p'],
        # and partner_em[511-p'] is exactly the complement of my tau=p.
        with ExitStack() as ph:
            big = ph.enter_context(tc.tile_pool(name="exbig", bufs=1))
            nc.sync.dma_start(emdr2[:], em_self[:, SB2:])
            rev2 = big.tile([T, SB2], BF16, tag="bigF", name="rev2")
            nc.sync.dma_start(
                rev2[:], emdr2[:].rearrange("j (t c) -> j t c",
                                            t=S2)[:, ::-1, :])
            slot = big.tile([T, SB2], F32, tag="bigB", name="slot")
            nc.vector.tensor_copy(slot[:], rev2[:])
            nc.sync.dma_start(emdb[:], slot[:])
            nc.gpsimd.collective_compute(
                "AllReduce", ALU.add,
                replica_groups=[[c, c + NPAIR] for c in range(NPAIR)],
                ins=[emdb.opt()], outs=[emdbo.opt()])
            t0 = big.tile([T, SB2], F32, tag="bigA", name="t0")
            nc.sync.dma_start(t0[:], emdbo[:])
            # em (first half, my orientation) = local + buf - rev2 + b_tag
            em1 = big.tile([T, SB2], F32, tag="bigD", name="em1")
            nc.vector.tensor_tensor(em1[:], t0[:], rev2[:], op=ALU.subtract)
            nc.vector.tensor_tensor(em1[:], em1[:], em_self[:, 0:SB2],
                                    op=ALU.add)
            nc.vector.tensor_scalar_add(em1[:], em1[:], tagb_sb[:])

            # ------------------------- CRF tail -------------------------
            sp = ph.enter_context(tc.tile_pool(name="crftmp", bufs=2))
            ap_ = ph.enter_context(tc.tile_pool(name="alphas", bufs=3))
            pp = ph.enter_context(tc.tile_pool(name="crfps", bufs=1,
                                               space="PSUM"))

            cst = sp.tile([T, T], F32, tag="cst")
            nc.sync.dma_start(cst[:], transR[:])
            cstT = sp.tile([T, T], F32, tag="cstT")
            nc.sync.dma_start(cstT[:], transRT[:])
            st_sb = sp.tile([T, 1], F32, tag="stv")
            nc.sync.dma_start(st_sb[:], svR[:])
            m0_sb = sp.tile([T, 1], F32, tag="m0v")
            nc.sync.dma_start(m0_sb[:], m0[:])
            m1_sb = sp.tile([T, 1], F32, tag="m1v")
            nc.sync.dma_start(m1_sb[:], m1[:])

            # numerator partial: my first-half em terms + start bonus +
            # transition pairs tau in [0,255], boundary pair half-weighted
            NLB = S2 + 1  # label steps needed: 0..256
            lab9 = big.tile([T, NLB * b], I32, tag="bigA", name="lab9")
            nc.sync.dma_start(
                lab9[:],
                labT[:].rearrange("s c -> (s c)")[None, 0:NLB * b]
                .broadcast_to((T, NLB * b)))
            io9 = sp.tile([T, 1], I32, tag="io9")
            nc.gpsimd.iota(io9[:], pattern=[[0, 1]], base=0,
                           channel_multiplier=1)
            io9f = sp.tile([T, 1], F32, tag="io9f")
            nc.gpsimd.tensor_copy(io9f[:], io9[:])
            labf = big.tile([T, NLB * b], F32, tag="bigC", name="labf")
            nc.gpsimd.tensor_copy(labf[:], lab9[:])
            onehot = big.tile([T, NLB * b], F32, tag="bigE", name="onehot")
            nc.gpsimd.tensor_scalar(onehot[:], labf[:], io9f[:], None,
                                    op0=ALU.is_equal)
            gmul = big.tile([T, SB2], F32, tag="bigA", name="gmul")
            nc.gpsimd.tensor_tensor(gmul[:], onehot[:, 0:SB2], em1[:],
                                    op=ALU.mult)
            acc = sp.tile([T, b], F32, tag="acc")
            nc.vector.tensor_reduce(
                acc[:], gmul[:].rearrange("j (t c) -> j c t", c=b),
                op=ALU.add, axis=AXL.X)
            stsc = sp.tile([T, b], F32, tag="stsc")
            nc.gpsimd.tensor_scalar_mul(stsc[:], onehot[:, 0:b], st_sb[:])
            nc.gpsimd.tensor_add(acc[:], acc[:], stsc[:])
            for tc0 in range(0, S2, 32):
                tn = min(32, S2 - tc0)
                thp = pp.tile([T, 32 * b], F32, tag="thp", bufs=2,
                              name=f"thp_{tc0}")
                nc.tensor.matmul(thp[:, 0:tn * b], cst[:],
                                 onehot[:, tc0 * b:(tc0 + tn) * b],
                                 start=True, stop=True)
                v = sp.tile([T, 32 * b], F32, tag="v")
                nc.vector.tensor_mul(v[:, 0:tn * b], thp[:, 0:tn * b],
                                     onehot[:, (tc0 + 1) * b:(tc0 + 1 + tn) * b])
                vr = sp.tile([T, b], F32, tag="vr")
                nc.vector.tensor_reduce(
                    vr[:], v[:, 0:tn * b].rearrange("j (t c) -> j c t", c=b),
                    op=ALU.add, axis=AXL.X)
                nc.vector.tensor_add(acc[:], acc[:], vr[:])
            # boundary pair (tau = S2-1) was fully counted on both cores:
            # subtract half of it here (symmetric on the pair)
            thb = pp.tile([T, b], F32, tag="thp", bufs=2, name="thb")
            nc.tensor.matmul(thb[:], cst[:],
                             onehot[:, (S2 - 1) * b:S2 * b],
                             start=True, stop=True)
            vb = sp.tile([T, b], F32, tag="vb")
            nc.vector.tensor_mul(vb[:], thb[:], onehot[:, S2 * b:(S2 + 1) * b])
            nc.vector.tensor_scalar_mul(vb[:], vb[:], -0.5)
            nc.vector.tensor_add(acc[:], acc[:], vb[:])
            ones9 = sp.tile([T, 1], F32, tag="ones9")
            nc.vector.memset(ones9[:], 1.0)
            ones19 = sp.tile([1, T], F32, tag="ones19")
            nc.vector.memset(ones19[:], 1.0)
            nump = pp.tile([1, b], F32, tag="ssum0", name="nump")
            nc.tensor.matmul(nump[:], ones9[:], acc[:], start=True, stop=True)
            num_sb = sp.tile([1, b], F32, tag="num")
            nc.vector.tensor_copy(num_sb[:], nump[:])

            # partition half-scan: probability domain, my orientation,
            # two interleaved 8-seq chains to pipeline PE<->DVE
            Em = sp.tile([T, T], F32, tag="Em")
            nc.scalar.activation(Em[:], cst[:], AFT.Exp)
            EmT = sp.tile([T, T], F32, tag="EmT")
            nc.scalar.activation(EmT[:], cstT[:], AFT.Exp)
            eem = big.tile([T, SB2], F32, tag="bigC", name="eem")
            nc.scalar.activation(eem[:], em1[:], AFT.Exp)
            es = sp.tile([T, 1], F32, tag="es")
            nc.scalar.activation(es[:], st_sb[:], AFT.Exp)
            logacc = sp.tile([1, b], F32, tag="logacc")
            nc.vector.memset(logacc[:], 0.0)
            alpha = ap_.tile([T, b], F32, tag="alpha", name="alpha_init")
            nc.vector.tensor_scalar_mul(alpha[:], eem[:, 0:b], es[:])
            for t in range(1, S2):
                aps = pp.tile([T, b], F32, tag="aps0", bufs=2,
                              name=f"apsn_{t}")
                nc.tensor.matmul(aps[:], Em[:], alpha[:], start=True, stop=True)
                alpha = ap_.tile([T, b], F32, tag="alpha", name=f"alpha_{t}")
                nc.vector.tensor_mul(alpha[:], aps[:], eem[:, t * b:(t + 1) * b])
                if t % 12 == 0 or t == S2 - 1:
                    ssum = pp.tile([1, b], F32, tag="ssum0", name=f"ss_{t}")
                    nc.tensor.matmul(ssum[:], ones9[:], alpha[:],
                                     start=True, stop=True)
                    ls = sp.tile([1, b], F32, tag="ls")
                    nc.scalar.activation(ls[:], ssum[:], AFT.Ln)
                    nc.vector.tensor_add(logacc[:], logacc[:], ls[:])
                    rc = sp.tile([1, b], F32, tag="rc")
                    nc.vector.reciprocal(rc[:], ssum[:])
                    bc = pp.tile([T, b], F32, tag="bc0", name=f"bc_{t}")
                    nc.tensor.matmul(bc[:], ones19[:], rc[:],
                                     start=True, stop=True)
                    a2 = ap_.tile([T, b], F32, tag="alpha", name=f"a2_{t}")
                    nc.vector.tensor_mul(a2[:], alpha[:], bc[:])
                    alpha = a2

            # ---------------- alpha exchange (pair AllReduce) ----------------
            # column-block layout (no partition shifts): cols [0:b] fwd
            # alpha, [b:2b] bwd alpha, [2b:3b] row0 = fwd logacc,
            # [3b:4b] row0 = bwd logacc
            slot4 = sp.tile([T, 4 * b], F32, tag="sl4")
            nc.vector.memset(slot4[:], 0.0)
            nc.vector.tensor_scalar_mul(slot4[:, 0:b], alpha[:], m0_sb[:])
            nc.vector.tensor_scalar_mul(slot4[:, b:2 * b], alpha[:], m1_sb[:])
            nc.vector.tensor_scalar_mul(slot4[0:1, 2 * b:3 * b], logacc[:],
                                        m0_sb[0:1, :])
            nc.vector.tensor_scalar_mul(slot4[0:1, 3 * b:4 * b], logacc[:],
                                        m1_sb[0:1, :])
            nc.sync.dma_start(albdb[:], slot4[:])
            nc.gpsimd.collective_compute(
                "AllReduce", ALU.add,
                replica_groups=[[c, c + NPAIR] for c in range(NPAIR)],
                ins=[albdb.opt()], outs=[albdbo.opt()])
            buf4 = sp.tile([T, 4 * b], F32, tag="bf4")
            nc.sync.dma_start(buf4[:], albdbo[:])
            # partner alpha/acc: pick the opposite role's column block
            pa = sp.tile([T, b], F32, tag="pa")
            nc.vector.tensor_scalar_mul(pa[:], buf4[:, 0:b], m1_sb[:])
            pb = sp.tile([T, b], F32, tag="pb")
            nc.vector.tensor_scalar_mul(pb[:], buf4[:, b:2 * b], m0_sb[:])
            nc.vector.tensor_add(pa[:], pa[:], pb[:])
            pacc = sp.tile([1, b], F32, tag="pacc")
            nc.vector.tensor_scalar_mul(pacc[:], buf4[0:1, 2 * b:3 * b],
                                        m1_sb[0:1, :])
            pacc2 = sp.tile([1, b], F32, tag="pacc2")
            nc.vector.tensor_scalar_mul(pacc2[:], buf4[0:1, 3 * b:4 * b],
                                        m0_sb[0:1, :])
            nc.vector.tensor_add(pacc[:], pacc[:], pacc2[:])

            # Z = alpha_mine^T . exp(transR) . alpha_partner
            vps = pp.tile([T, b], F32, tag="bc0", name="vps")
            nc.tensor.matmul(vps[:], EmT[:], pa[:], start=True, stop=True)
            wz = sp.tile([T, b], F32, tag="wz")
            nc.vector.tensor_mul(wz[:], alpha[:], vps[:])
            zp = pp.tile([1, b], F32, tag="ssum0", name="zp")
            nc.tensor.matmul(zp[:], ones9[:], wz[:], start=True, stop=True)
            lz = sp.tile([1, b], F32, tag="lz")
            nc.scalar.activation(lz[:], zp[:], AFT.Ln)
            logz = sp.tile([1, b], F32, tag="logz")
            nc.vector.tensor_add(logz[:], lz[:], logacc[:])
            nc.vector.tensor_add(logz[:], logz[:], pacc[:])
            nc.vector.tensor_scalar_mul(logz[:], logz[:], 0.5)
            dbg_sb = sp.tile([1, 4 * b], F32, tag="dbgt")
            nc.vector.tensor_copy(dbg_sb[:, 0:b], num_sb[:])
            nc.vector.tensor_copy(dbg_sb[:, b:2 * b], lz[:])
            nc.vector.tensor_copy(dbg_sb[:, 2 * b:3 * b], logacc[:])
            nc.vector.tensor_copy(dbg_sb[:, 3 * b:4 * b], pacc[:])
            nc.sync.dma_start(dbg[:].rearrange("r c -> (r c)")[None, :],
                              dbg_sb[:])
            lv = sp.tile([1, b], F32, tag="lv")
            nc.vector.tensor_sub(lv[:], num_sb[:], logz[:])
            tot = sp.tile([1, 1], F32, tag="tot")
            nc.vector.tensor_reduce(tot[:], lv[:], op=ALU.add, axis=AXL.X)
            sc = sp.tile([1, 1], F32, tag="sc")
            nc.vector.tensor_scalar_mul(sc[:], tot[:], -1.0 / B_full)
            nc.sync.dma_start(lossdb[:], sc[:])
            nc.gpsimd.collective_compute(
                "AllReduce", ALU.add,
                replica_groups=[list(range(NCORES))],
                ins=[lossdb.opt()], outs=[lossout.opt()])
            lf = sp.tile([1, 1], F32, tag="lf")
            nc.sync.dma_start(lf[:], lossout[:])
            nc.sync.dma_start(loss[:], lf[:])

    nc.compile()
    return nc


# ---------------------------------------------------------------------------
# host-side sharding
# ---------------------------------------------------------------------------

def _perm_khalf(HD):
    """Gate-dim perm: torch (i,f,g,o) -> khalf-major (i,f,o,g)x2 halves."""
    base = {"i": 0, "f": HD, "g": 2 * HD, "o": 3 * HD}
    idx = []
    half = HD // 2
    for H in range(2):
        for gate in ("i", "f", "o", "g"):
            lo = base[gate] + H * half
            idx.append(np.arange(lo, lo + half))
    return np.concatenate(idx)


def shard_inputs(inputs, b, S, E, HD, T):
    KE, NH = E // 128, HD // 128
    NM = 4 * NH
    T1 = T + 1
    perm = _perm_khalf(HD)
    gmask = np.zeros(4 * HD, np.float32)  # rows holding g-gate blocks
    half = HD // 2
    for H in range(2):
        lo = (H * 4 + 3) * half
        gmask[lo:lo + half] = 1.0
    bf = ml_dtypes.bfloat16
    f8 = ml_dtypes.float8_e4m3fn
    f8e3 = ml_dtypes.float8_e3m4
    KE2 = KE // 2
    x = np.asarray(inputs["x"], np.float32)[:, :S]
    labels = np.asarray(inputs["labels"]).astype(np.int32)[:, :S]
    trans = np.asarray(inputs["transitions"], np.float32)
    startv = np.asarray(inputs["start_trans"], np.float32).reshape(T, 1)
    endv = np.asarray(inputs["end_trans"], np.float32).reshape(T, 1)
    Wtag = np.asarray(inputs["W_tag"], np.float32)
    btag = np.asarray(inputs["b_tag"], np.float32).reshape(T, 1)

    per_dir = {}
    for d, sfx in enumerate(("f", "b")):
        Wih = np.asarray(inputs[f"W_ih_{sfx}"], np.float32)[perm].copy()
        Whh = np.asarray(inputs[f"W_hh_{sfx}"], np.float32)[perm].copy()
        bias = (np.asarray(inputs[f"b_ih_{sfx}"], np.float32)
                + np.asarray(inputs[f"b_hh_{sfx}"], np.float32))[perm].copy()
        scale = 1.0 + gmask  # 2x on g rows (tanh -> sigmoid fold)
        Wih *= scale[:, None]
        Whh *= scale[:, None] * WHH_SCL
        bias *= scale
        per_dir[d] = dict(
            wihT=np.ascontiguousarray(
                Wih.T.reshape(KE2, 2, 128, 4 * HD).transpose(0, 2, 1, 3)
            ).astype(f8),
            whhT=np.ascontiguousarray(
                Whh.T.reshape(NH, 128, 4 * HD)).astype(f8e3),
            bias4=np.ascontiguousarray(
                bias.reshape(NM, 128).T).astype(np.float32),
            wtagT=np.ascontiguousarray(
                Wtag[:, d * HD:(d + 1) * HD].T.reshape(NH, 128, T)).astype(bf),
            transR=(trans if d == 0
                    else np.ascontiguousarray(trans.T)),
            transRT=(np.ascontiguousarray(trans.T) if d == 0 else trans),
            svR=startv if d == 0 else endv,
            m0=np.full((T, 1), 1.0 - d, np.float32),
            m1=np.full((T, 1), float(d), np.float32),
        )

    in_maps = []
    for c in range(NCORES):
        d = c // NPAIR
        g = c % NPAIR
        xs = x[g * b:(g + 1) * b]
        lab = labels[g * b:(g + 1) * b]
        if d == 1:
            xs = xs[:, ::-1, :]
            lab = lab[:, ::-1]
        m = dict(per_dir[d])
        m["xT"] = np.ascontiguousarray(
            xs.transpose(2, 1, 0).reshape(KE2, 2, 128, S * b)
            .transpose(0, 2, 1, 3)).astype(f8)
        m["labT"] = np.ascontiguousarray(lab.T)
        m["tagb"] = btag
        in_maps.append(m)
    return in_maps


# ---------------------------------------------------------------------------
# entry point
# ---------------------------------------------------------------------------

_B, _S, _E, _HD, _T = 64, 512, 1024, 512, 9
_cache = {}


def _get_program(S=_S):
    if S not in _cache:
        _cache[S] = build_program(_B // NPAIR, S, _E, _HD, _T, _B)
    return _cache[S]


def kernel(**inputs) -> np.ndarray:
    from concourse.bass_utils import run_bass_kernel_spmd
    nc = _get_program()
    in_maps = shard_inputs(inputs, _B // NPAIR, _S, _E, _HD, _T)
    res = run_bass_kernel_spmd(nc, in_maps, list(range(NCORES)))
    out = np.asarray(res.results[0]["loss"], np.float32).reshape(())
    return out


# revision 31
# speedup vs baseline: 1.1127x; 1.0133x over previous
"""BiLSTM-CRF loss kernel for Trainium2 (8 NeuronCores, Bass/Tile) — v9.

Pair scheme: cores 0-3 run the FWD direction for seqs [16c:16c+16],
cores 4-7 the BWD direction for the same seqs (host feeds time-reversed
x / labels / role-swapped CRF tensors, so the program is SPMD-uniform).

Changes vs the v3 baseline:
 - Whh stationary is fp8e3m4 (host-prescaled x32 on top of the x2
   g-row tanh fold) with h kept bf16 (mixed-dtype matmul): FWL loads a
   128-col fp8 stationary in ~32 cycles vs 64 for bf16, and the
   recurrence is LDWEIGHTS-bound at N=16 — measured 25 ns/MM.
 - No GX injection matmul: the gate PSUM holds pure 32*Whh.h and a DVE
   scalar_tensor_tensor computes gp/32 + gx in place before the
   sigmoid.
 - Each gate half accumulates into its own full PSUM bank so one
   half's PSUM reads overlap the other half's matmul stream.
 - The input-projection (a_work) jobs drain at the END of each step so
   their 700 ns DVE bias-adds stop head-of-line-blocking the cell
   chain on the DVE FIFO, paced on a cumulative schedule so the PE
   filler lasts the whole 32-step window.
 - CRF scan renorm stays at R=12: the ACT engine's Ln spline table
   saturates for inputs around e^40+, so prob-domain sums must stay
   well below that (R=20 silently lost ~2.3 nats per renorm).
 - CRF alpha/beta split: each core scans only 255 steps of its own
   orientation (fwd = alpha half, bwd = beta half via the transposed
   recursion), exchanges only the second half of its emissions plus
   the final alpha/logacc, and the pair combines
   Z = alpha_255^T E alphahat_255.  Numerator is split between the
   pair (each side takes its first-half em terms + its start bonus;
   the shared boundary transition is half-weighted on both).
"""

import sys

sys.path.insert(0, "/opt/trn_rl_repo")

import numpy as np
import ml_dtypes
from contextlib import ExitStack

import concourse.bass as bass
import concourse.bacc as bacc
import concourse.tile as tile
import concourse.mybir as mybir

F32 = mybir.dt.float32
BF16 = mybir.dt.bfloat16
FP8 = mybir.dt.float8e4
FP8E3 = mybir.dt.float8e3
I32 = mybir.dt.int32
PM = mybir.MatmulPerfMode
AFT = mybir.ActivationFunctionType
ALU = mybir.AluOpType
AXL = mybir.AxisListType

NCORES = 8
NPAIR = 4
WHH_SCL = 32.0  # host prescale on Whh (fp8e3m4 grid); DVE divides back


def build_program(b, S, E, HD, T, B_full, R=20, AW=32, ECH=16):
    KE = E // 128
    NH = HD // 128          # 4 k-tiles of h
    NM = 4 * NH             # 16 gate m-tiles, khalf-major perm order
    W = NH * b              # h width per step (64 for b=16)
    WH = W // 2             # per-half h width (32)
    SB = S * b
    S2 = S // 2
    SB2 = S2 * b
    H4 = 4 * HD
    NWIN = S // AW
    T1 = T + 1              # alpha + logacc rows
    assert S % AW == 0 and AW % ECH == 0

    nc = bacc.Bacc("TRN2", target_bir_lowering=False, debug=False,
                   num_devices=NCORES)

    KE2 = KE // 2
    xT = nc.dram_tensor("xT", [KE2, 128, 2, SB], FP8, kind="ExternalInput")
    wihT = nc.dram_tensor("wihT", [KE2, 128, 2, H4], FP8, kind="ExternalInput")
    whhT = nc.dram_tensor("whhT", [NH, 128, H4], FP8E3, kind="ExternalInput")
    eye = nc.dram_tensor("eye", [128, 128], BF16, kind="ExternalInput")
    bias4 = nc.dram_tensor("bias4", [128, NM], F32, kind="ExternalInput")
    wtagT = nc.dram_tensor("wtagT", [NH, 128, T], BF16, kind="ExternalInput")
    tagb = nc.dram_tensor("tagb", [T, 1], F32, kind="ExternalInput")
    labT = nc.dram_tensor("labT", [S, b], I32, kind="ExternalInput")
    transR = nc.dram_tensor("transR", [T, T], F32, kind="ExternalInput")
    transRT = nc.dram_tensor("transRT", [T, T], F32, kind="ExternalInput")
    svR = nc.dram_tensor("svR", [T, 1], F32, kind="ExternalInput")
    m0 = nc.dram_tensor("m0", [T, 1], F32, kind="ExternalInput")
    m1 = nc.dram_tensor("m1", [T, 1], F32, kind="ExternalInput")
    loss = nc.dram_tensor("loss", [1, 1], F32, kind="ExternalOutput")
    dbg = nc.dram_tensor("dbg", [4, b], F32, kind="ExternalOutput")

    with tile.TileContext(nc) as tc, ExitStack() as top:
        dram = top.enter_context(tc.tile_pool(name="dram", bufs=1, space="DRAM"))
        emdr2 = dram.tile([T, SB2], BF16)
        emdb = dram.tile([T, SB2], F32)
        emdbo = dram.tile([T, SB2], F32)
        albdb = dram.tile([T, 4 * b], F32)
        albdbo = dram.tile([T, 4 * b], F32)
        lossdb = dram.tile([1, 1], F32)
        lossout = dram.tile([1, 1], F32)

        persist = top.enter_context(tc.tile_pool(name="persist", bufs=1))
        wtag_sb = persist.tile([128, NH * T], BF16)
        nc.sync.dma_start(wtag_sb[:], wtagT[:])
        eye_sb = persist.tile([128, 128], BF16)
        nc.sync.dma_start(eye_sb[:], eye[:])
        tagb_sb = persist.tile([T, 1], F32)
        nc.sync.dma_start(tagb_sb[:], tagb[:])
        em_self = persist.tile([T, SB], BF16)

        # ------------------------- fused LSTM loop -------------------------
        lstm = ExitStack()
        wpool = lstm.enter_context(tc.tile_pool(name="wpool", bufs=1))
        wih_sb = wpool.tile([128, KE2 * 2 * H4], FP8)
        nc.sync.dma_start(wih_sb[:], wihT[:])
        whh_sb = wpool.tile([128, NH * H4], FP8E3)
        nc.sync.dma_start(whh_sb[:], whhT[:])
        bias_sb = wpool.tile([128, NM], F32)
        nc.sync.dma_start(bias_sb[:], bias4[:])

        xwin = lstm.enter_context(tc.tile_pool(name="xwin", bufs=2))
        gxwin = lstm.enter_context(tc.tile_pool(name="gxwin", bufs=2))
        apsum = lstm.enter_context(tc.tile_pool(name="apsum", bufs=2,
                                                space="PSUM"))
        # one full PSUM bank per (half, parity) so PSUM reads of one
        # half overlap the other half's matmul stream
        gpsumA = lstm.enter_context(tc.tile_pool(name="gpsumA", bufs=2,
                                                 space="PSUM"))
        gpsumB = lstm.enter_context(tc.tile_pool(name="gpsumB", bufs=2,
                                                 space="PSUM"))
        empsum = lstm.enter_context(tc.tile_pool(name="empsum", bufs=2,
                                                 space="PSUM"))
        hpool = lstm.enter_context(tc.tile_pool(name="hpool", bufs=2))
        spool = lstm.enter_context(tc.tile_pool(name="spool", bufs=2))
        cpool = lstm.enter_context(tc.tile_pool(name="cpool", bufs=1))
        tpool = lstm.enter_context(tc.tile_pool(name="tpool", bufs=2))

        ct = [cpool.tile([128, WH], F32, tag=f"c{h}", name=f"ct{h}")
              for h in range(2)]

        def dma_xwin(w):
            t = xwin.tile([128, KE2 * 2 * AW * b], FP8, tag="xw",
                          name=f"xw_{w}")
            nc.sync.dma_start(t[:],
                             xT[:, :, :, w * AW * b:(w + 1) * AW * b])
            return t

        def a_work(w, xw, gxt):
            wv = wih_sb[:].rearrange("p (k q c) -> p k q c", k=KE2, q=2)
            xv = xw[:].rearrange("p (k q c) -> p k q c", k=KE2, q=2)
            for mm in range(NM):
                ap = apsum.tile([128, AW * b], F32, tag="aps",
                                name=f"aps_{w}_{mm}")
                for ke in range(KE2):
                    yield lambda ap=ap, mm=mm, ke=ke, xv=xv, wv=wv: \
                        nc.tensor.matmul(
                            ap[:],
                            wv[:, ke, :, mm * 128:(mm + 1) * 128],
                            xv[:, ke, :, :],
                            start=(ke == 0), stop=(ke == KE2 - 1),
                            perf_mode=PM.DoubleRow)
                yield lambda ap=ap, mm=mm, gxt=gxt: \
                    nc.vector.tensor_scalar_add(
                        gxt[:].rearrange("p (t m c) -> p t m c",
                                         t=AW, m=NM)[:, :, mm, :],
                        ap[:].rearrange("p (t c) -> p t c", t=AW),
                        bias_sb[:, mm:mm + 1])

        xw_t = dma_xwin(0)
        gx_t = gxwin.tile([128, NM * AW * b], BF16, tag="gx", name="gx_0")
        for job in a_work(0, xw_t, gx_t):
            job()
        xw_next = gx_next = None

        pending = []
        npend0 = 0
        drained = 0
        h_prev = None
        hch = None
        GB = 8 * b  # gate cols per half (128)
        for tau in range(S):
            w, sw = tau // AW, tau % AW
            if sw == 0:
                if w + 1 < NWIN:
                    xw_next = dma_xwin(w + 1)
                    gx_next = gxwin.tile([128, NM * AW * b], BF16,
                                         tag="gx", name=f"gx_{w + 1}")
                    pending = list(a_work(w + 1, xw_next, gx_next))
                    pending.reverse()
                    npend0 = len(pending)
                    drained = 0
                else:
                    pending = []
                    npend0 = drained = 0
            # cumulative pacing: by end of step sw, (sw+1)/AW of the jobs
            quota = (-(-npend0 * (sw + 1) // AW) - drained) if pending else 0

            gxv = gx_t[:].rearrange("p (t m c) -> p t m c",
                                    t=AW, m=NM)[:, sw, :, :]
            if tau % ECH == 0:
                hch = hpool.tile([128, ECH * W], BF16, tag="h",
                                 name=f"hch_{tau}")
            toff = (tau % ECH) * W
            for H in range(2):
                pool = gpsumA if H == 0 else gpsumB
                gxh = gxv[:, H * 8:(H + 1) * 8, :]
                sg = spool.tile([128, GB], F32, tag=f"sig{H}",
                                name=f"sg_{H}_{tau}")
                if tau == 0:
                    nc.scalar.activation(
                        sg[:].rearrange("p (m c) -> p m c", m=8),
                        gxh, AFT.Sigmoid)
                else:
                    gp = pool.tile([128, 512], F32, tag=f"g{H}",
                                   name=f"gp_{H}_{tau}")
                    # PE: inject GX (1x), then 32*Whh.(h/32) recurrence
                    nc.tensor.matmul(
                        gp[:, 0:GB].rearrange("p (m c) -> p m c", m=8),
                        eye_sb[:], gxh,
                        start=True, stop=False, skip_group_check=True)
                    for kh in range(2):          # input k-half
                        for mm in range(H * 8, (H + 1) * 8):
                            for kt in (2 * kh, 2 * kh + 1):
                                mloc = mm - H * 8
                                nc.tensor.matmul(
                                    gp[:, mloc * b:(mloc + 1) * b],
                                    whh_sb[:, kt * H4 + mm * 128:
                                           kt * H4 + (mm + 1) * 128],
                                    h_prev[:, kt * b:(kt + 1) * b],
                                    start=False,
                                    stop=(mm == (H + 1) * 8 - 1 and
                                          kt == 2 * kh + 1),
                                    skip_group_check=True)
                    nc.scalar.activation(sg[:], gp[:, 0:GB], AFT.Sigmoid)

                def _v2(out):
                    # fused (sig_g2 - 0.5) * sig_i always on DVE (Pool
                    # lacks scalar_tensor_tensor and is slow on 2-op seqs)
                    nc.vector.scalar_tensor_tensor(
                        out, sg[:, 3 * WH:4 * WH], 0.5, sg[:, 0:WH],
                        op0=ALU.subtract, op1=ALU.mult)
                if tau == 0:
                    _v2(ct[H][:])
                else:
                    v2 = tpool.tile([128, WH], F32, tag=f"v2{H}",
                                    name=f"v2_{H}_{tau}")
                    _v2(v2[:])
                    wt = tpool.tile([128, WH], F32, tag=f"w{H}",
                                    name=f"wt_{H}_{tau}")
                    nc.vector.tensor_tensor(wt[:], sg[:, WH:2 * WH],
                                            ct[H][:], op=ALU.mult)
                    nc.vector.tensor_tensor(ct[H][:], v2[:], wt[:],
                                            op=ALU.add)
                tca = tpool.tile([128, WH], F32, tag=f"tanc{H}",
                                 name=f"tca_{H}_{tau}")
                nc.scalar.activation(tca[:], ct[H][:], AFT.Tanh, scale=2.0)
                # h half H -> k-tiles 2H,2H+1 = cols [H*WH : H*WH+WH]
                nc.vector.scalar_tensor_tensor(
                    hch[:, toff + H * WH:toff + (H + 1) * WH],
                    sg[:, 2 * WH:3 * WH], 1.0 / WHH_SCL, tca[:],
                    op0=ALU.mult, op1=ALU.mult)
            h_prev = hch[:, toff:toff + W]
            # a_work jobs drain at end of step so their DVE ops land
            # behind the cell chain (DVE queue is strict FIFO)
            for _ in range(quota):
                if pending:
                    pending.pop()()
                    drained += 1

            if tau % ECH == ECH - 1:
                ep = empsum.tile([T, ECH * b], F32, tag="emps",
                                 name=f"ep_{tau}")
                hv = hch[:].rearrange("p (t k c) -> p t k c", t=ECH, k=NH)
                for kt in range(NH):
                    nc.tensor.matmul(
                        ep[:].rearrange("p (t c) -> p t c", t=ECH),
                        wtag_sb[:, kt * T:(kt + 1) * T], hv[:, :, kt, :],
                        start=(kt == 0), stop=(kt == NH - 1))
                emv = em_self[:].rearrange(
                    "j (t c) -> j t c", t=S)[:, tau - ECH + 1:tau + 1, :]
                nc.vector.tensor_copy(
                    emv, ep[:].rearrange("p (t c) -> p t c", t=ECH))
            if sw == AW - 1 and gx_next is not None:
                xw_t, gx_t = xw_next, gx_next
                xw_next = gx_next = None

        lstm.close()

        # ---------------- em exchange (pair AllReduce, half S) --------------
        # Each core ships its SECOND-half-of-tau emissions reversed into
        # p-order; buf[p] then holds my_em[511-p] + partner_em[511-p'],
        # and partner_em[511-p'] is exactly the complement of my tau=p.
        with ExitStack() as ph:
            ph.enter_context(nc.allow_low_precision(
                reason="bf16 CRF scan; log-acc and renorm sums stay f32"))
            big = ph.enter_context(tc.tile_pool(name="exbig", bufs=1))
            nc.sync.dma_start(emdr2[:], em_self[:, SB2:])
            rev2 = big.tile([T, SB2], BF16, tag="bigF", name="rev2")
            nc.sync.dma_start(
                rev2[:], emdr2[:].rearrange("j (t c) -> j t c",
                                            t=S2)[:, ::-1, :])
            slot = big.tile([T, SB2], F32, tag="bigB", name="slot")
            nc.vector.tensor_copy(slot[:], rev2[:])
            nc.sync.dma_start(emdb[:], slot[:])
            nc.gpsimd.collective_compute(
                "AllReduce", ALU.add,
                replica_groups=[[c, c + NPAIR] for c in range(NPAIR)],
                ins=[emdb.opt()], outs=[emdbo.opt()])
            t0 = big.tile([T, SB2], F32, tag="bigA", name="t0")
            nc.sync.dma_start(t0[:], emdbo[:])
            # em (first half, my orientation) = local + buf - rev2 + b_tag
            em1 = big.tile([T, SB2], F32, tag="bigD", name="em1")
            nc.vector.tensor_tensor(em1[:], t0[:], rev2[:], op=ALU.subtract)
            nc.vector.tensor_tensor(em1[:], em1[:], em_self[:, 0:SB2],
                                    op=ALU.add)
            nc.vector.tensor_scalar_add(em1[:], em1[:], tagb_sb[:])

            # ------------------------- CRF tail -------------------------
            sp = ph.enter_context(tc.tile_pool(name="crftmp", bufs=2))
            ap_ = ph.enter_context(tc.tile_pool(name="alphas", bufs=3))
            pp = ph.enter_context(tc.tile_pool(name="crfps", bufs=1,
                                               space="PSUM"))

            cst = sp.tile([T, T], F32, tag="cst")
            nc.sync.dma_start(cst[:], transR[:])
            cstT = sp.tile([T, T], F32, tag="cstT")
            nc.sync.dma_start(cstT[:], transRT[:])
            st_sb = sp.tile([T, 1], F32, tag="stv")
            nc.sync.dma_start(st_sb[:], svR[:])
            m0_sb = sp.tile([T, 1], F32, tag="m0v")
            nc.sync.dma_start(m0_sb[:], m0[:])
            m1_sb = sp.tile([T, 1], F32, tag="m1v")
            nc.sync.dma_start(m1_sb[:], m1[:])

            # numerator partial: my first-half em terms + start bonus +
            # transition pairs tau in [0,255], boundary pair half-weighted
            NLB = S2 + 1  # label steps needed: 0..256
            lab9 = big.tile([T, NLB * b], I32, tag="bigA", name="lab9")
            nc.sync.dma_start(
                lab9[:],
                labT[:].rearrange("s c -> (s c)")[None, 0:NLB * b]
                .broadcast_to((T, NLB * b)))
            io9 = sp.tile([T, 1], I32, tag="io9")
            nc.gpsimd.iota(io9[:], pattern=[[0, 1]], base=0,
                           channel_multiplier=1)
            io9f = sp.tile([T, 1], F32, tag="io9f")
            nc.gpsimd.tensor_copy(io9f[:], io9[:])
            labf = big.tile([T, NLB * b], F32, tag="bigC", name="labf")
            nc.gpsimd.tensor_copy(labf[:], lab9[:])
            onehot = big.tile([T, NLB * b], F32, tag="bigE", name="onehot")
            nc.gpsimd.tensor_scalar(onehot[:], labf[:], io9f[:], None,
                                    op0=ALU.is_equal)
            gmul = big.tile([T, SB2], F32, tag="bigA", name="gmul")
            nc.gpsimd.tensor_tensor(gmul[:], onehot[:, 0:SB2], em1[:],
                                    op=ALU.mult)
            acc = sp.tile([T, b], F32, tag="acc")
            nc.vector.tensor_reduce(
                acc[:], gmul[:].rearrange("j (t c) -> j c t", c=b),
                op=ALU.add, axis=AXL.X)
            stsc = sp.tile([T, b], F32, tag="stsc")
            nc.gpsimd.tensor_scalar_mul(stsc[:], onehot[:, 0:b], st_sb[:])
            nc.gpsimd.tensor_add(acc[:], acc[:], stsc[:])
            for tc0 in range(0, S2, 32):
                tn = min(32, S2 - tc0)
                thp = pp.tile([T, 32 * b], F32, tag="thp", bufs=2,
                              name=f"thp_{tc0}")
                nc.tensor.matmul(thp[:, 0:tn * b], cst[:],
                                 onehot[:, tc0 * b:(tc0 + tn) * b],
                                 start=True, stop=True)
                v = sp.tile([T, 32 * b], F32, tag="v")
                nc.vector.tensor_mul(v[:, 0:tn * b], thp[:, 0:tn * b],
                                     onehot[:, (tc0 + 1) * b:(tc0 + 1 + tn) * b])
                vr = sp.tile([T, b], F32, tag="vr")
                nc.vector.tensor_reduce(
                    vr[:], v[:, 0:tn * b].rearrange("j (t c) -> j c t", c=b),
                    op=ALU.add, axis=AXL.X)
                nc.vector.tensor_add(acc[:], acc[:], vr[:])
            # boundary pair (tau = S2-1) was fully counted on both cores:
            # subtract half of it here (symmetric on the pair)
            thb = pp.tile([T, b], F32, tag="thp", bufs=2, name="thb")
            nc.tensor.matmul(thb[:], cst[:],
                             onehot[:, (S2 - 1) * b:S2 * b],
                             start=True, stop=True)
            vb = sp.tile([T, b], F32, tag="vb")
            nc.vector.tensor_mul(vb[:], thb[:], onehot[:, S2 * b:(S2 + 1) * b])
            nc.vector.tensor_scalar_mul(vb[:], vb[:], -0.5)
            nc.vector.tensor_add(acc[:], acc[:], vb[:])
            ones9 = sp.tile([T, 1], BF16, tag="ones9")
            nc.vector.memset(ones9[:], 1.0)
            ones19 = sp.tile([1, T], BF16, tag="ones19")
            nc.vector.memset(ones19[:], 1.0)
            accb = sp.tile([T, b], BF16, tag="accb")
            nc.vector.tensor_copy(accb[:], acc[:])
            nump = pp.tile([1, b], F32, tag="ssum0", name="nump")
            nc.tensor.matmul(nump[:], ones9[:], accb[:], start=True, stop=True)
            num_sb = sp.tile([1, b], F32, tag="num")
            nc.vector.tensor_copy(num_sb[:], nump[:])

            # partition half-scan: probability domain, my orientation,
            # two interleaved 8-seq chains to pipeline PE<->DVE
            Em = sp.tile([T, T], BF16, tag="Em")
            nc.scalar.activation(Em[:], cst[:], AFT.Exp)
            EmT = sp.tile([T, T], BF16, tag="EmT")
            nc.scalar.activation(EmT[:], cstT[:], AFT.Exp)
            eem = big.tile([T, SB2], F32, tag="bigC", name="eem")
            nc.scalar.activation(eem[:], em1[:], AFT.Exp)
            es = sp.tile([T, 1], F32, tag="es")
            nc.scalar.activation(es[:], st_sb[:], AFT.Exp)
            logacc = sp.tile([1, b], F32, tag="logacc")
            nc.vector.memset(logacc[:], 0.0)
            alpha = ap_.tile([T, b], BF16, tag="alpha", name="alpha_init")
            nc.vector.tensor_scalar_mul(alpha[:], eem[:, 0:b], es[:])
            for t in range(1, S2):
                aps = pp.tile([T, b], F32, tag="aps0", bufs=2,
                              name=f"apsn_{t}")
                nc.tensor.matmul(aps[:], Em[:], alpha[:], start=True, stop=True)
                alpha = ap_.tile([T, b], BF16, tag="alpha", name=f"alpha_{t}")
                nc.vector.tensor_mul(alpha[:], aps[:], eem[:, t * b:(t + 1) * b])
                if t % 12 == 0 or t == S2 - 1:
                    ssum = pp.tile([1, b], F32, tag="ssum0", name=f"ss_{t}")
                    nc.tensor.matmul(ssum[:], ones9[:], alpha[:],
                                     start=True, stop=True)
                    ls = sp.tile([1, b], F32, tag="ls")
                    nc.scalar.activation(ls[:], ssum[:], AFT.Ln)
                    nc.vector.tensor_add(logacc[:], logacc[:], ls[:])
                    rc = sp.tile([1, b], BF16, tag="rc")
                    nc.vector.reciprocal(rc[:], ssum[:])
                    bc = pp.tile([T, b], F32, tag="bc0", name=f"bc_{t}")
                    nc.tensor.matmul(bc[:], ones19[:], rc[:],
                                     start=True, stop=True)
                    a2 = ap_.tile([T, b], BF16, tag="alpha", name=f"a2_{t}")
                    nc.vector.tensor_mul(a2[:], alpha[:], bc[:])
                    alpha = a2

            # ---------------- alpha exchange (pair AllReduce) ----------------
            # column-block layout (no partition shifts): cols [0:b] fwd
            # alpha, [b:2b] bwd alpha, [2b:3b] row0 = fwd logacc,
            # [3b:4b] row0 = bwd logacc
            slot4 = sp.tile([T, 4 * b], F32, tag="sl4")
            nc.vector.memset(slot4[:], 0.0)
            nc.vector.tensor_scalar_mul(slot4[:, 0:b], alpha[:], m0_sb[:])
            nc.vector.tensor_scalar_mul(slot4[:, b:2 * b], alpha[:], m1_sb[:])
            nc.vector.tensor_scalar_mul(slot4[0:1, 2 * b:3 * b], logacc[:],
                                        m0_sb[0:1, :])
            nc.vector.tensor_scalar_mul(slot4[0:1, 3 * b:4 * b], logacc[:],
                                        m1_sb[0:1, :])
            nc.sync.dma_start(albdb[:], slot4[:])
            nc.gpsimd.collective_compute(
                "AllReduce", ALU.add,
                replica_groups=[[c, c + NPAIR] for c in range(NPAIR)],
                ins=[albdb.opt()], outs=[albdbo.opt()])
            buf4 = sp.tile([T, 4 * b], F32, tag="bf4")
            nc.sync.dma_start(buf4[:], albdbo[:])
            # partner alpha/acc: pick the opposite role's column block
            pa = sp.tile([T, b], BF16, tag="pa")
            nc.vector.tensor_scalar_mul(pa[:], buf4[:, 0:b], m1_sb[:])
            pb = sp.tile([T, b], BF16, tag="pb")
            nc.vector.tensor_scalar_mul(pb[:], buf4[:, b:2 * b], m0_sb[:])
            nc.vector.tensor_add(pa[:], pa[:], pb[:])
            pacc = sp.tile([1, b], F32, tag="pacc")
            nc.vector.tensor_scalar_mul(pacc[:], buf4[0:1, 2 * b:3 * b],
                                        m1_sb[0:1, :])
            pacc2 = sp.tile([1, b], F32, tag="pacc2")
            nc.vector.tensor_scalar_mul(pacc2[:], buf4[0:1, 3 * b:4 * b],
                                        m0_sb[0:1, :])
            nc.vector.tensor_add(pacc[:], pacc[:], pacc2[:])

            # Z = alpha_mine^T . exp(transR) . alpha_partner
            vps = pp.tile([T, b], F32, tag="bc0", name="vps")
            nc.tensor.matmul(vps[:], EmT[:], pa[:], start=True, stop=True)
            wz = sp.tile([T, b], BF16, tag="wz")
            nc.vector.tensor_mul(wz[:], alpha[:], vps[:])
            zp = pp.tile([1, b], F32, tag="ssum0", name="zp")
            nc.tensor.matmul(zp[:], ones9[:], wz[:], start=True, stop=True)
            lz = sp.tile([1, b], F32, tag="lz")
            nc.scalar.activation(lz[:], zp[:], AFT.Ln)
            logz = sp.tile([1, b], F32, tag="logz")
            nc.vector.tensor_add(logz[:], lz[:], logacc[:])
            nc.vector.tensor_add(logz[:], logz[:], pacc[:])
            nc.vector.tensor_scalar_mul(logz[:], logz[:], 0.5)
            dbg_sb = sp.tile([1, 4 * b], F32, tag="dbgt")
            nc.vector.tensor_copy(dbg_sb[:, 0:b], num_sb[:])
            nc.vector.tensor_copy(dbg_sb[:, b:2 * b], lz[:])
            nc.vector.tensor_copy(dbg_sb[:, 2 * b:3 * b], logacc[:])
            nc.vector.tensor_copy(dbg_sb[:, 3 * b:4 * b], pacc[:])
            nc.sync.dma_start(dbg[:].rearrange("r c -> (r c)")[None, :],
                              dbg_sb[:])
            lv = sp.tile([1, b], F32, tag="lv")
            nc.vector.tensor_sub(lv[:], num_sb[:], logz[:])
            tot = sp.tile([1, 1], F32, tag="tot")
            nc.vector.tensor_reduce(tot[:], lv[:], op=ALU.add, axis=AXL.X)
            sc = sp.tile([1, 1], F32, tag="sc")
            nc.vector.tensor_scalar_mul(sc[:], tot[:], -1.0 / B_full)
            nc.sync.dma_start(lossdb[:], sc[:])
            nc.gpsimd.collective_compute(
                "AllReduce", ALU.add,
                replica_groups=[list(range(NCORES))],
                ins=[lossdb.opt()], outs=[lossout.opt()])
            lf = sp.tile([1, 1], F32, tag="lf")
            nc.sync.dma_start(lf[:], lossout[:])
            nc.sync.dma_start(loss[:], lf[:])

    nc.compile()
    return nc


# ---------------------------------------------------------------------------
# host-side sharding
# ---------------------------------------------------------------------------

def _perm_khalf(HD):
    """Gate-dim perm: torch (i,f,g,o) -> khalf-major (i,f,o,g)x2 halves."""
    base = {"i": 0, "f": HD, "g": 2 * HD, "o": 3 * HD}
    idx = []
    half = HD // 2
    for H in range(2):
        for gate in ("i", "f", "o", "g"):
            lo = base[gate] + H * half
            idx.append(np.arange(lo, lo + half))
    return np.concatenate(idx)


def shard_inputs(inputs, b, S, E, HD, T):
    KE, NH = E // 128, HD // 128
    NM = 4 * NH
    T1 = T + 1
    perm = _perm_khalf(HD)
    gmask = np.zeros(4 * HD, np.float32)  # rows holding g-gate blocks
    half = HD // 2
    for H in range(2):
        lo = (H * 4 + 3) * half
        gmask[lo:lo + half] = 1.0
    bf = ml_dtypes.bfloat16
    f8 = ml_dtypes.float8_e4m3fn
    f8e3 = ml_dtypes.float8_e3m4
    KE2 = KE // 2
    x = np.asarray(inputs["x"], np.float32)[:, :S]
    labels = np.asarray(inputs["labels"]).astype(np.int32)[:, :S]
    trans = np.asarray(inputs["transitions"], np.float32)
    startv = np.asarray(inputs["start_trans"], np.float32).reshape(T, 1)
    endv = np.asarray(inputs["end_trans"], np.float32).reshape(T, 1)
    Wtag = np.asarray(inputs["W_tag"], np.float32)
    btag = np.asarray(inputs["b_tag"], np.float32).reshape(T, 1)

    per_dir = {}
    for d, sfx in enumerate(("f", "b")):
        Wih = np.asarray(inputs[f"W_ih_{sfx}"], np.float32)[perm].copy()
        Whh = np.asarray(inputs[f"W_hh_{sfx}"], np.float32)[perm].copy()
        bias = (np.asarray(inputs[f"b_ih_{sfx}"], np.float32)
                + np.asarray(inputs[f"b_hh_{sfx}"], np.float32))[perm].copy()
        scale = 1.0 + gmask  # 2x on g rows (tanh -> sigmoid fold)
        Wih *= scale[:, None]
        Whh *= scale[:, None] * WHH_SCL
        bias *= scale
        per_dir[d] = dict(
            wihT=np.ascontiguousarray(
                Wih.T.reshape(KE2, 2, 128, 4 * HD).transpose(0, 2, 1, 3)
            ).astype(f8),
            whhT=np.ascontiguousarray(
                Whh.T.reshape(NH, 128, 4 * HD)).astype(f8e3),
            bias4=np.ascontiguousarray(
                bias.reshape(NM, 128).T).astype(np.float32),
            wtagT=np.ascontiguousarray(
                (Wtag[:, d * HD:(d + 1) * HD] * WHH_SCL).T
                .reshape(NH, 128, T)).astype(bf),
            transR=(trans if d == 0
                    else np.ascontiguousarray(trans.T)),
            transRT=(np.ascontiguousarray(trans.T) if d == 0 else trans),
            svR=startv if d == 0 else endv,
            m0=np.full((T, 1), 1.0 - d, np.float32),
            m1=np.full((T, 1), float(d), np.float32),
        )

    in_maps = []
    for c in range(NCORES):
        d = c // NPAIR
        g = c % NPAIR
        xs = x[g * b:(g + 1) * b]
        lab = labels[g * b:(g + 1) * b]
        if d == 1:
            xs = xs[:, ::-1, :]
            lab = lab[:, ::-1]
        m = dict(per_dir[d])
        m["xT"] = np.ascontiguousarray(
            xs.transpose(2, 1, 0).reshape(KE2, 2, 128, S * b)
            .transpose(0, 2, 1, 3)).astype(f8)
        m["labT"] = np.ascontiguousarray(lab.T)
        m["tagb"] = btag
        m["eye"] = np.eye(128, dtype=np.float32).astype(bf)
        in_maps.append(m)
    return in_maps


# ---------------------------------------------------------------------------
# entry point
# ---------------------------------------------------------------------------

_B, _S, _E, _HD, _T = 64, 512, 1024, 512, 9
_cache = {}


def _get_program(S=_S):
    if S not in _cache:
        _cache[S] = build_program(_B // NPAIR, S, _E, _HD, _T, _B)
    return _cache[S]


def kernel(**inputs) -> np.ndarray:
    from concourse.bass_utils import run_bass_kernel_spmd
    nc = _get_program()
    in_maps = shard_inputs(inputs, _B // NPAIR, _S, _E, _HD, _T)
    res = run_bass_kernel_spmd(nc, in_maps, list(range(NCORES)))
    out = np.asarray(res.results[0]["loss"], np.float32).reshape(())
    return out
